# revision 1
# baseline (speedup 1.0000x reference)
"""Trainium2 Bass kernel for equivariant multihead attention.

Math (per batch b, query point i, coset s1, channel c):
    logit[j,s2] = sum_g pairwise_g[b,i,j,s1,s2,g]*w_g[c,g]
                  + w_y[c,0]*y[b,j,s2,c] + w_y[c,1]*y[b,i,s1,c] + b_g[c] + b_y[c]
    att = exp(logit)*mask[b,j,s2];  att /= sum_{j,s2} att
    out = (y[b,i,s1,c] + sum_{j,s2} att*y[b,j,s2,c]) * mask[b,i,s1]  @ w_lin.T

The query-side term and the biases are constant over the key dims (j,s2), so
they cancel in the normalization and are dropped.  The key-side factor
exp(w_y[c,0]*y[b,j,s2,c])*mask[b,j,s2] is a tiny per-batch table KD (and
KD*y = KN), precomputed on host.  Per (b,i) block the device computes
    E[(s1,s2,c), j] = exp(sum_g G_T[(s1,s2,g), j] * w_g[c,g])
    den_part[(s1,s2,c)] = sum_j E * KD_T     (fused multiply-reduce)
    num_part[(s1,s2,c)] = sum_j E * KN_T
and one final PE matmul sums the partials over s2.  Host finishes with the
residual add, query mask, and the c_in->c_out linear (all tiny).

Sharding: query dim i is split 8 ways (16 i x 4 b = 64 blocks per core).

Implementation notes (walrus on this stack allows only ONE sync wait per
Matmult / DMA / STT instruction, and ~12 on the final drain):
  * ALL inputs ship as ONE dram "blob" per core, loaded by 7 big
    column-range DMAs into a single SBUF plane -> every DMA is the first
    on its HW-DGE queue (no proc-predecessor wait) and descriptor runs are
    ~17KB contiguous (max DMA efficiency).  The final store is the 8th DMA
    (queue 7, also virgin).
  * tiny "spacer" ops make each engine observe cross-engine ticks ahead of
    the real instructions, so those carry at most one wait each.
"""

import numpy as np

import concourse.bacc as bacc
import concourse.tile as tile
from concourse import mybir
from concourse.bass_utils import run_bass_kernel_spmd

B, N, S, CIN, COUT, GDIM = 4, 128, 8, 8, 8, 7
NCORES = 8
ISHARD = N // NCORES          # 16 query points per core
NBLK = B * ISHARD             # 64 (b,i) blocks per core
PW = S * GDIM                 # 56: free width of one s1 slice
QW = 2 * PW                   # 112: free width of one transpose quarter
NQ = 4                        # quarters per block
BW = NQ * QW                  # 448 floats per (j, block)
NCOL = NBLK * NQ              # 256 partial columns per half

# blob column layout: [ident | kd | kn | bd | sind | pg blocks]
IDENT0 = 0
KD0 = 128
KN0 = KD0 + B * N             # 640
BD0 = KN0 + B * N             # 1152
SIND0 = BD0 + 128             # 1280
CONSTW = SIND0 + 16           # 1296
TOTW = CONSTW + NBLK * BW     # 29968

# blocks covered by each of the 7 input DMAs (first also carries consts;
# earlier ones smaller for a faster pipeline ramp)
SUPER_BLOCKS = (4, 6, 8, 10, 11, 12, 13)

F32 = mybir.dt.float32

# per-quarter engine assignment (balance tuning): PSUM->SBUF copy of the
# transposed quarter, and den/num fused multiply-reduces.  bacc's
# split_sync_waits legalizes any multi-wait instructions this creates.
# (gpsimd cannot run scalar_tensor_tensor: not a valid Pool-engine opcode)
COPY_ENG = ("act", "act", "act", "dve")
DEN_ENG = ("dve", "dve", "dve", "dve")
NUM_ENG = ("dve", "dve", "dve", "dve")

_PROGRAM_CACHE = {}


def _build_program(nblk=NBLK, loop_reps=1):
    """loop_reps>1 wraps the main loop in a hardware For_i that re-runs the
    full pass (including the input DMAs) on the same data -- used only for
    timing: wall(loop_reps=R) - wall(loop_reps=1) isolates device time from
    the ~100ms axon dispatch/transfer overhead."""
    nc = bacc.Bacc("TRN2", target_bir_lowering=False, debug=False,
                   num_devices=NCORES)

    blob_d = nc.dram_tensor("blob", (N, TOTW), F32, kind="ExternalInput").ap()
    out_s = nc.dram_tensor("out_s", (16, 2 * NCOL), F32,
                           kind="ExternalOutput").ap()

    # per-super [start_block, end_block) and column ranges
    supers = []
    blk0 = 0
    for nb in SUPER_BLOCKS:
        if blk0 >= nblk:
            break
        nb = min(nb, nblk - blk0)
        c0 = 0 if blk0 == 0 else CONSTW + blk0 * BW
        c1 = CONSTW + (blk0 + nb) * BW
        supers.append((blk0, blk0 + nb, c0, c1))
        blk0 += nb

    with tile.TileContext(nc) as tc:
        with (
            tc.tile_pool(name="consts", bufs=1) as consts,
            tc.tile_pool(name="gtpool", bufs=4) as gtpool,
            tc.tile_pool(name="epool", bufs=4) as epool,
            tc.tile_pool(name="psA", bufs=4, space="PSUM") as psA,
            tc.tile_pool(name="psB", bufs=2, space="PSUM") as psB,
            tc.tile_pool(name="psC", bufs=1, space="PSUM") as psC,
        ):
            g_all = consts.tile([N, TOTW], F32)
            ident = g_all[:, IDENT0:IDENT0 + 128]
            bd = g_all[0:QW, BD0:BD0 + 128]
            sind = g_all[:, SIND0:SIND0 + 16]

            buf_dve = consts.tile([128, 2 * NCOL], F32)
            nc.vector.memset(buf_dve, 0.0)

            NDUM = 8
            dummies = [consts.tile([128, 1], F32, name=f"dum{i}")
                       for i in range(NDUM)]
            dum_idx = [0]
            s_sb = consts.tile([16, 2 * NCOL], F32)

            def stt_reduce(eng, e_q, table, col_ap):
                dum = dummies[dum_idx[0] % NDUM]
                dum_idx[0] += 1
                engine = nc.vector if eng == "dve" else nc.gpsimd
                engine.scalar_tensor_tensor(
                    dum.broadcast_to(e_q.shape), e_q, 0.0, table,
                    op0=mybir.AluOpType.bypass, op1=mybir.AluOpType.mult,
                    accum_out=col_ap)

            def main_pass():
              for (b0, b1, c0, c1) in supers:
                nc.sync.dma_start(g_all[:, c0:c1], blob_d[:, c0:c1])
              for (b0, b1, c0, c1) in supers:
                for blk in range(b0, b1):
                    b = blk // ISHARD
                    gcol = CONSTW + blk * BW
                    kd_b = g_all[:, KD0 + b * N:KD0 + (b + 1) * N]
                    kn_b = g_all[:, KN0 + b * N:KN0 + (b + 1) * N]

                    gt_cat = gtpool.tile([QW, NQ, 128], F32, tag="gt")
                    for q in range(NQ):
                        gt_ps = psA.tile([QW, 128], F32, tag="gtps")
                        nc.tensor.transpose(
                            gt_ps,
                            g_all[:, gcol + QW * q:gcol + QW * (q + 1)],
                            ident)
                        if COPY_ENG[q] == "act":
                            nc.scalar.copy(gt_cat[:, q, :], gt_ps)
                        else:
                            nc.vector.tensor_copy(gt_cat[:, q, :], gt_ps)

                    l_ps = psB.tile([128, NQ, 128], F32, tag="lps")
                    nc.tensor.matmul(l_ps, lhsT=bd, rhs=gt_cat,
                                     start=True, stop=True)

                    e_t = epool.tile([128, NQ, 128], F32, tag="e")
                    nc.scalar.activation(e_t, l_ps,
                                         mybir.ActivationFunctionType.Exp)

                    for q in range(NQ):
                        col = blk * NQ + q
                        e_q = e_t[:, q, :]
                        stt_reduce(DEN_ENG[q], e_q, kd_b,
                                   buf_dve[:, col:col + 1])
                        stt_reduce(NUM_ENG[q], e_q, kn_b,
                                   buf_dve[:, NCOL + col:NCOL + col + 1])

            if loop_reps > 1:
                with tc.For_i(0, loop_reps, 1,
                              hint_engines=(mybir.EngineType.PE,
                                            mybir.EngineType.Activation,
                                            mybir.EngineType.DVE,
                                            mybir.EngineType.SP)):
                    main_pass()
            else:
                main_pass()

            # sum the (h,s2,c) j-partials over s2 -> (h,c)
            s_ps = psC.tile([16, 2 * NCOL], F32)
            nc.tensor.matmul(s_ps, lhsT=sind, rhs=buf_dve,
                             start=True, stop=True)
            nc.scalar.copy(s_sb, s_ps)
            nc.sync.dma_start(out_s, s_sb)   # 8th DMA -> virgin queue 7

    nc.compile()   # bacc: register alloc + split_sync_waits (1-wait limit)
    return nc


def _get_program(nblk=NBLK, loop_reps=1):
    key = ("nc", nblk, loop_reps)
    if key not in _PROGRAM_CACHE:
        _PROGRAM_CACHE[key] = _build_program(nblk, loop_reps)
    return _PROGRAM_CACHE[key]


def _host_prep(pairwise_g, coset_functions, mask, w_y, w_g):
    """Build the per-core input blobs."""
    y = coset_functions.astype(np.float32)          # (B, N, S, C) keys
    maskf = mask.astype(np.float32)
    ey = np.exp(y * w_y[:, 0]) * maskf[..., None]   # (B, j, s2, c)
    kn = ey * y
    # rows (h, s2, c) with h in {0,1} duplicated; cols j
    kd_t = np.tile(ey.transpose(0, 2, 3, 1).reshape(B, S * CIN, N), (1, 2, 1))
    kn_t = np.tile(kn.transpose(0, 2, 3, 1).reshape(B, S * CIN, N), (1, 2, 1))

    bd = np.zeros((128, 128), np.float32)
    for pl in range(16):
        for g in range(GDIM):
            for c in range(CIN):
                bd[pl * GDIM + g, pl * CIN + c] = w_g[c, g]

    sind = np.zeros((128, 16), np.float32)
    for h in range(2):
        for s2 in range(S):
            for c in range(CIN):
                sind[h * 64 + s2 * CIN + c, h * CIN + c] = 1.0

    consts_plane = np.empty((N, CONSTW), np.float32)
    consts_plane[:, IDENT0:IDENT0 + 128] = np.eye(128, dtype=np.float32)
    consts_plane[:, KD0:KD0 + B * N] = kd_t.transpose(1, 0, 2).reshape(128, -1)
    consts_plane[:, KN0:KN0 + B * N] = kn_t.transpose(1, 0, 2).reshape(128, -1)
    consts_plane[:, BD0:BD0 + 128] = bd
    consts_plane[:, SIND0:SIND0 + 16] = sind

    in_maps = []
    for k in range(NCORES):
        sl = slice(ISHARD * k, ISHARD * (k + 1))
        pg_core = pairwise_g[:, sl].reshape(NBLK, N, BW)
        blob = np.empty((N, TOTW), np.float32)
        blob[:, :CONSTW] = consts_plane
        blob[:, CONSTW:] = pg_core.transpose(1, 0, 2).reshape(N, NBLK * BW)
        in_maps.append({"blob": blob})
    return in_maps


def _host_finish(s_list, coset_functions, mask, w_lin):
    """Decode per-core (16, 512) outputs into the full result."""
    y = np.asarray(coset_functions, dtype=np.float32)
    maskf = np.asarray(mask).astype(np.float32)
    out = np.empty((B, N, S, COUT), np.float32)
    for k in range(NCORES):
        s = s_list[k]
        den = s[:, :NCOL].reshape(2, CIN, NBLK, NQ)
        num = s[:, NCOL:].reshape(2, CIN, NBLK, NQ)
        # (h, c, blk, q) -> (blk, s1 = 2q + h, c)
        den = den.transpose(2, 3, 0, 1).reshape(NBLK, S, CIN)
        num = num.transpose(2, 3, 0, 1).reshape(NBLK, S, CIN)
        sl = slice(ISHARD * k, ISHARD * (k + 1))
        y_q = y[:, sl].reshape(NBLK, S, CIN)
        m_q = maskf[:, sl].reshape(NBLK, S)
        res = (y_q + num / den) * m_q[..., None]
        res = res @ w_lin.T
        out[:, sl] = res.reshape(B, ISHARD, S, COUT)
    return out


def kernel(pairwise_g, coset_functions, mask, w_y, b_y, w_g, b_g, w_lin):
    pairwise_g = np.asarray(pairwise_g, dtype=np.float32)
    coset_functions = np.asarray(coset_functions, dtype=np.float32)
    mask = np.asarray(mask)
    w_y = np.asarray(w_y, dtype=np.float32)
    w_g = np.asarray(w_g, dtype=np.float32)
    w_lin = np.asarray(w_lin, dtype=np.float32)

    nc = _get_program()
    in_maps = _host_prep(pairwise_g, coset_functions, mask, w_y, w_g)
    res = run_bass_kernel_spmd(nc, in_maps, core_ids=list(range(NCORES)))
    s_list = [r["out_s"] for r in res.results]
    return _host_finish(s_list, coset_functions, mask, w_lin)



# revision 2
# speedup vs baseline: 4.6662x; 4.6662x over previous
"""Trainium2 Bass kernel for equivariant multihead attention (v2).

Math (per batch b, query point i, coset s1, channel c):
    logit[j,s2] = sum_g pairwise_g[b,i,j,s1,s2,g]*w_g[c,g]
                  + w_y[c,0]*y[b,j,s2,c] + w_y[c,1]*y[b,i,s1,c] + b_g[c] + b_y[c]
    att = exp(logit)*mask[b,j,s2];  att /= sum_{j,s2} att
    out = (y[b,i,s1,c] + sum_{j,s2} att*y[b,j,s2,c]) * mask[b,i,s1]  @ w_lin.T

Query-side terms and biases are constant over key dims (j,s2) -> cancel in the
normalization.  The key-side term + log-mask are folded INTO the logit matmul:
the contraction dim (s1,s2b,g) uses only 112 of 128 partitions, and the key
term  w0[c]*y[b,j,s2,c] + logmask[b,j,s2]  is constant over s1, so it is an
exact rank-16 function of ((c,s2b) x (s2a,j)) -- carried on the 16 spare
contraction rows (indicator columns in the weight, key-table rows under each
G^T block).  Then

    E[(s1,c,s2b), (s2a,j)] = exp(matmul)          # includes key factor + mask
    den[(s1,c,s2b)] = sum_{s2a,j} E               # FREE: activation accum_out
    num[(s1,c,s2b)] = sum_{s2a,j} E * ytbl        # ONE full-width DVE STT

Per (b,i) block: 1 bf16 PE matmul (128-contraction, stationary weights,
512 rows = ~213ns), 1 ScalarE exp psum->sbuf with accum, 1 DVE STT (bf16 2x
mode).  Host sums the s2b pairs, divides, adds residual, applies query mask
and the c_in->c_out linear (all tiny).

Sharding: query dim i is split 8 ways (16 i x 4 b = 64 blocks per core).
All input ships as ONE bf16 dram blob per core via 7 column-range DMAs; the
fp32 [128,128] den/num buffer is the single output DMA (queue 8).
"""

import numpy as np
import ml_dtypes

import concourse.bacc as bacc
import concourse.tile as tile
from concourse import mybir
from concourse.bass_utils import run_bass_kernel_spmd

B, N, S, CIN, COUT, GDIM = 4, 128, 8, 8, 8, 7
NCORES = 8
ISHARD = N // NCORES          # 16 query points per core
NBLK = B * ISHARD             # 64 (b,i) blocks per core
BW = 4 * N                    # 512: free width of one block stripe (s2a, j)
KROWS = S * 2 * GDIM          # 112 contraction rows (s1, s2b, g)

# blob column layout (bf16): [bd_aug | ytbl (4 batches) | G stripes]
BD0 = 0
YT0 = 128
G0 = YT0 + B * BW             # 2176
TOTW = G0 + NBLK * BW         # 34944

# blocks covered by each of the 7 input DMAs (first also carries consts;
# earlier ones smaller for a faster pipeline ramp)
SUPER_BLOCKS = (2, 4, 6, 10, 12, 14, 16)

F32 = mybir.dt.float32
BF16 = mybir.dt.bfloat16
NPBF16 = ml_dtypes.bfloat16

LOGMASK0 = -30.0              # logit offset for masked keys: exp(-30) ~ 1e-13

_PROGRAM_CACHE = {}


def _build_program(nblk=NBLK, loop_reps=1):
    """loop_reps>1 wraps the main loop in a hardware For_i that re-runs the
    full pass (including the input DMAs) on the same data -- used only for
    timing: wall(loop_reps=R) - wall(loop_reps=1) isolates device time from
    the ~100ms axon dispatch/transfer overhead."""
    nc = bacc.Bacc("TRN2", target_bir_lowering=False, debug=False,
                   num_devices=NCORES)

    blob_d = nc.dram_tensor("blob", (128, TOTW), BF16, kind="ExternalInput").ap()
    out_s = nc.dram_tensor("out_s", (128, 2 * NBLK), F32,
                           kind="ExternalOutput").ap()

    # per-super [start_block, end_block) and column ranges
    supers = []
    blk0 = 0
    for nb in SUPER_BLOCKS:
        if blk0 >= nblk:
            break
        nb = min(nb, nblk - blk0)
        c0 = 0 if blk0 == 0 else G0 + blk0 * BW
        c1 = G0 + (blk0 + nb) * BW
        supers.append((blk0, blk0 + nb, c0, c1))
        blk0 += nb

    with tile.TileContext(nc) as tc:
        with (
            tc.tile_pool(name="consts", bufs=1) as consts,
            tc.tile_pool(name="epool", bufs=4) as epool,
            tc.tile_pool(name="psA", bufs=4, space="PSUM") as psA,
        ):
            g_all = consts.tile([128, TOTW], BF16)
            bd_aug = g_all[:, BD0:BD0 + 128]
            buf = consts.tile([128, 2 * NBLK], F32)
            nc.vector.memset(buf, 0.0)

            def main_pass():
                for (b0, b1, c0, c1) in supers:
                    nc.sync.dma_start(g_all[:, c0:c1], blob_d[:, c0:c1])
                for (b0, b1, c0, c1) in supers:
                    for blk in range(b0, b1):
                        b = blk // ISHARD
                        rhs = g_all[:, G0 + blk * BW:G0 + (blk + 1) * BW]
                        l_ps = psA.tile([128, BW], F32, tag="lps")
                        nc.tensor.matmul(l_ps, lhsT=bd_aug, rhs=rhs,
                                         start=True, stop=True)
                        e_t = epool.tile([128, BW], BF16, tag="e")
                        nc.scalar.activation(
                            e_t, l_ps, mybir.ActivationFunctionType.Exp,
                            accum_out=buf[:, 2 * blk:2 * blk + 1])
                        scr = epool.tile([128, BW], BF16, tag="scr")
                        nc.vector.scalar_tensor_tensor(
                            scr, e_t, 0.0,
                            g_all[:, YT0 + b * BW:YT0 + (b + 1) * BW],
                            op0=mybir.AluOpType.bypass,
                            op1=mybir.AluOpType.mult,
                            accum_out=buf[:, 2 * blk + 1:2 * blk + 2])

            if loop_reps > 1:
                with tc.For_i(0, loop_reps, 1,
                              hint_engines=(mybir.EngineType.PE,
                                            mybir.EngineType.Activation,
                                            mybir.EngineType.DVE,
                                            mybir.EngineType.SP)):
                    main_pass()
            else:
                main_pass()

            nc.sync.dma_start(out_s, buf)   # 8th DMA -> virgin queue 7

    nc.compile()   # bacc: register alloc + split_sync_waits (1-wait limit)
    return nc


def _get_program(nblk=NBLK, loop_reps=1):
    key = ("nc", nblk, loop_reps)
    if key not in _PROGRAM_CACHE:
        _PROGRAM_CACHE[key] = _build_program(nblk, loop_reps)
    return _PROGRAM_CACHE[key]


def _host_prep(pairwise_g, coset_functions, mask, w_y, w_g):
    """Build the per-core bf16 input blobs."""
    y = coset_functions.astype(np.float32)          # (B, N, S, C) keys
    logmask = np.where(mask, 0.0, LOGMASK0).astype(np.float32)  # (B, N, S)
    w0 = w_y[:, 0].astype(np.float32)               # (CIN,)

    # bd_aug [128, 128]: col m = (s1, c, s2b) = s1*16 + c*2 + s2b
    #   rows 0..111: k = (s1', s2b', g) -> w_g[c, g] iff s1'==s1, s2b'==s2b
    #   rows 112..127: k = 112 + (c', s2b') -> 1 iff c'==c, s2b''==s2b
    bd = np.zeros((128, 128), np.float32)
    for s1 in range(S):
        for s2b in range(2):
            for g in range(GDIM):
                row = s1 * 14 + s2b * 7 + g
                for c in range(CIN):
                    bd[row, s1 * 16 + c * 2 + s2b] = w_g[c, g]
    for c in range(CIN):
        for s2b in range(2):
            row = 112 + c * 2 + s2b
            for s1 in range(S):
                bd[row, s1 * 16 + c * 2 + s2b] = 1.0

    # per-batch tables, cols (s2a, j) = s2a*128 + j
    # y[b] (j, s2, c) -> (c, s2b, s2a, j)
    y_t = y.transpose(0, 3, 2, 1).reshape(B, CIN, 2, 4, N)      # b,c,s2b,s2a,j
    # ytbl [128, 512]: row (s1, c, s2b), replicated over s1
    ytbl = np.broadcast_to(y_t.reshape(B, 1, CIN, 2, 4 * N),
                           (B, S, CIN, 2, 4 * N)).reshape(B, 128, BW)
    # keytbl [16, 512]: row (c, s2b): w0[c]*y + logmask
    lm_t = logmask.transpose(0, 2, 1).reshape(B, 1, 2, 4, N)    # b,1,s2b,s2a,j
    kt = (w0[None, :, None, None, None] * y_t + lm_t).reshape(B, 16, BW)

    # G^T per block [112, 512]: row (s1, s2b, g), col (s2a, j)
    in_maps = []
    for k in range(NCORES):
        sl = slice(ISHARD * k, ISHARD * (k + 1))
        pg = pairwise_g[:, sl]                      # (B, 16, N, S, S, G)
        arr = pg.reshape(B, ISHARD, N, S, 2, 4, GDIM)
        arr = arr.transpose(0, 1, 3, 4, 6, 5, 2)    # b,i,s1,s2b,g,s2a,j
        arr = arr.reshape(NBLK, KROWS, BW)

        blob = np.empty((128, TOTW), NPBF16)
        blob[:, BD0:BD0 + 128] = bd.astype(NPBF16)
        blob[:, YT0:G0] = ytbl.transpose(1, 0, 2).reshape(128, B * BW)
        gdst = blob[:, G0:].reshape(128, NBLK, BW)
        gdst[:KROWS] = arr.transpose(1, 0, 2)
        gdst[KROWS:] = np.repeat(kt, ISHARD, axis=0).transpose(1, 0, 2)
        in_maps.append({"blob": blob})
    return in_maps


def _host_finish(s_list, coset_functions, mask, w_lin):
    """Decode per-core (128, 128) den/num buffers into the full result."""
    y = np.asarray(coset_functions, dtype=np.float32)
    maskf = np.asarray(mask).astype(np.float32)
    out = np.empty((B, N, S, COUT), np.float32)
    for k in range(NCORES):
        s = s_list[k].astype(np.float32)            # [128, 2*NBLK]
        # rows p = (s1, c, s2b); col 2*blk = den, 2*blk+1 = num
        den = s[:, 0::2].reshape(S, CIN, 2, NBLK).sum(axis=2)   # (s1, c, blk)
        num = s[:, 1::2].reshape(S, CIN, 2, NBLK).sum(axis=2)
        ratio = (num / den).transpose(2, 0, 1)      # (blk, s1, c)
        sl = slice(ISHARD * k, ISHARD * (k + 1))
        y_q = y[:, sl].reshape(NBLK, S, CIN)
        m_q = maskf[:, sl].reshape(NBLK, S)
        res = (y_q + ratio) * m_q[..., None]
        res = res @ w_lin.T
        out[:, sl] = res.reshape(B, ISHARD, S, COUT)
    return out


def kernel(pairwise_g, coset_functions, mask, w_y, b_y, w_g, b_g, w_lin):
    pairwise_g = np.asarray(pairwise_g, dtype=np.float32)
    coset_functions = np.asarray(coset_functions, dtype=np.float32)
    mask = np.asarray(mask)
    w_y = np.asarray(w_y, dtype=np.float32)
    w_g = np.asarray(w_g, dtype=np.float32)
    w_lin = np.asarray(w_lin, dtype=np.float32)

    nc = _get_program()
    in_maps = _host_prep(pairwise_g, coset_functions, mask, w_y, w_g)
    res = run_bass_kernel_spmd(nc, in_maps, core_ids=list(range(NCORES)))
    s_list = [r["out_s"] for r in res.results]
    return _host_finish(s_list, coset_functions, mask, w_lin)


# revision 7
# speedup vs baseline: 11.2615x; 2.4134x over previous
"""Trainium2 Bass kernel for equivariant multihead attention (v2).

Math (per batch b, query point i, coset s1, channel c):
    logit[j,s2] = sum_g pairwise_g[b,i,j,s1,s2,g]*w_g[c,g]
                  + w_y[c,0]*y[b,j,s2,c] + w_y[c,1]*y[b,i,s1,c] + b_g[c] + b_y[c]
    att = exp(logit)*mask[b,j,s2];  att /= sum_{j,s2} att
    out = (y[b,i,s1,c] + sum_{j,s2} att*y[b,j,s2,c]) * mask[b,i,s1]  @ w_lin.T

Query-side terms and biases are constant over key dims (j,s2) -> cancel in the
normalization.  The key-side term + log-mask are folded INTO the logit matmul:
the contraction dim (s1,s2b,g) uses only 112 of 128 partitions, and the key
term  w0[c]*y[b,j,s2,c] + logmask[b,j,s2]  is constant over s1, so it is an
exact rank-16 function of ((c,s2b) x (s2a,j)) -- carried on the 16 spare
contraction rows (indicator columns in the weight, key-table rows under each
G^T block).  Then

    E[(s1,c,s2b), (s2a,j)] = exp(matmul)          # includes key factor + mask
    den[(s1,c,s2b)] = sum_{s2a,j} E               # FREE: activation accum_out
    num[(s1,c,s2b)] = sum_{s2a,j} E * ytbl        # ONE full-width DVE STT

Per (b,i) block: 1 bf16 PE matmul (128-contraction, stationary weights,
512 rows = ~213ns), 1 ScalarE exp psum->sbuf with accum, 1 DVE STT (bf16 2x
mode).  Host sums the s2b pairs, divides, adds residual, applies query mask
and the c_in->c_out linear (all tiny).

Sharding: query dim i is split 8 ways (16 i x 4 b = 64 blocks per core).
All input ships as ONE bf16 dram blob per core via 7 column-range DMAs; the
fp32 [128,128] den/num buffer is the single output DMA (queue 8).
"""

import numpy as np
import ml_dtypes

import concourse.bacc as bacc
import concourse.tile as tile
from concourse import mybir
from concourse.bass_utils import run_bass_kernel_spmd

B, N, S, CIN, COUT, GDIM = 4, 128, 8, 8, 8, 7
NCORES = 8
ISHARD = N // NCORES          # 16 query points per core
NBLK = B * ISHARD             # 64 (b,i) blocks per core
BW = 4 * N                    # 512: free width of one block stripe (s2a, j)
KROWS = S * 2 * GDIM          # 112 contraction rows (s1, s2b, g)

# blob column layout (bf16): [bd_aug | ytbl (4 batches) | G stripes]
BD0 = 0
YT0 = 128
G0 = YT0 + B * BW             # 2176
TOTW = G0 + NBLK * BW         # 34944

# blocks covered by each of the 7 input DMAs (first also carries consts;
# earlier ones smaller for a faster pipeline ramp)
SUPER_BLOCKS = (2, 4, 6, 10, 12, 14, 16)

F32 = mybir.dt.float32
BF16 = mybir.dt.bfloat16
NPBF16 = ml_dtypes.bfloat16

LOGMASK0 = -30.0              # logit offset for masked keys: exp(-30) ~ 1e-13

_PROGRAM_CACHE = {}


def _build_program(nblk=NBLK, loop_reps=1, mode="full"):
    """loop_reps>1 wraps the main loop in a hardware For_i that re-runs the
    full pass (including the input DMAs) on the same data -- used only for
    timing: wall(loop_reps=R) - wall(loop_reps=1) isolates device time from
    the ~100ms axon dispatch/transfer overhead.

    mode: subtractive-profiling variants ("full", "no_stt", "mm_only",
    "dma_only", "no_mm")."""
    nc = bacc.Bacc("TRN2", target_bir_lowering=False, debug=False,
                   num_devices=NCORES)

    blob_d = nc.dram_tensor("blob", (128, TOTW), BF16, kind="ExternalInput").ap()
    out_s = nc.dram_tensor("out_s", (128, 2 * NBLK), F32,
                           kind="ExternalOutput").ap()

    # per-super [start_block, end_block) and column ranges
    supers = []
    blk0 = 0
    for nb in SUPER_BLOCKS:
        if blk0 >= nblk:
            break
        nb = min(nb, nblk - blk0)
        c0 = 0 if blk0 == 0 else G0 + blk0 * BW
        c1 = G0 + (blk0 + nb) * BW
        supers.append((blk0, blk0 + nb, c0, c1))
        blk0 += nb

    with tile.TileContext(nc) as tc:
        with (
            tc.tile_pool(name="consts", bufs=1) as consts,
            tc.tile_pool(name="epool", bufs=4) as epool,
            tc.tile_pool(name="psA", bufs=4, space="PSUM") as psA,
        ):
            g_all = consts.tile([128, TOTW], BF16)
            bd_aug = g_all[:, BD0:BD0 + 128]
            buf = consts.tile([128, 2 * NBLK], F32)
            nc.vector.memset(buf, 0.0)
            # unique exp-output buffer per block: the activation then waits
            # ONLY on its matmul (1 sync wait -> no ScalarE spacer ops)
            e_all = consts.tile([128, NBLK, BW], BF16)

            def main_pass():
                for (b0, b1, c0, c1) in supers:
                    nc.sync.dma_start(g_all[:, c0:c1], blob_d[:, c0:c1])
                if mode == "dma_only":
                    nc.scalar.copy(buf[:, 0:1], g_all[:, 0:1])
                    return
                for (b0, b1, c0, c1) in supers:
                    for blk in range(b0, b1):
                        b = blk // ISHARD
                        rhs = g_all[:, G0 + blk * BW:G0 + (blk + 1) * BW]
                        if mode != "no_mm":
                            l_ps = psA.tile([128, BW], F32, tag="lps")
                            nc.tensor.matmul(l_ps, lhsT=bd_aug, rhs=rhs,
                                             start=True, stop=True)
                            if mode == "mm_only":
                                continue
                            e_src = l_ps
                        else:
                            e_src = rhs
                        e_t = e_all[:, blk, :]
                        nc.scalar.activation(
                            e_t, e_src, mybir.ActivationFunctionType.Exp,
                            accum_out=buf[:, 2 * blk:2 * blk + 1])
                        if mode == "no_stt":
                            continue
                        scr = epool.tile([128, BW], BF16, tag="scr")
                        nc.vector.scalar_tensor_tensor(
                            scr, e_t, 0.0,
                            g_all[:, YT0 + b * BW:YT0 + (b + 1) * BW],
                            op0=mybir.AluOpType.bypass,
                            op1=mybir.AluOpType.mult,
                            accum_out=buf[:, 2 * blk + 1:2 * blk + 2])

            if loop_reps > 1:
                with tc.For_i(0, loop_reps, 1,
                              hint_engines=(mybir.EngineType.PE,
                                            mybir.EngineType.Activation,
                                            mybir.EngineType.DVE,
                                            mybir.EngineType.SP)):
                    main_pass()
            else:
                main_pass()

            nc.sync.dma_start(out_s, buf)   # 8th DMA -> virgin queue 7

    nc.compile()   # bacc: register alloc + split_sync_waits (1-wait limit)
    return nc


def _get_program(nblk=NBLK, loop_reps=1, mode="full"):
    key = ("nc", nblk, loop_reps, mode)
    if key not in _PROGRAM_CACHE:
        _PROGRAM_CACHE[key] = _build_program(nblk, loop_reps, mode)
    return _PROGRAM_CACHE[key]


def _host_prep(pairwise_g, coset_functions, mask, w_y, w_g):
    """Build the per-core bf16 input blobs."""
    y = coset_functions.astype(np.float32)          # (B, N, S, C) keys
    logmask = np.where(mask, 0.0, LOGMASK0).astype(np.float32)  # (B, N, S)
    w0 = w_y[:, 0].astype(np.float32)               # (CIN,)

    # bd_aug [128, 128]: col m = (s1, c, s2b) = s1*16 + c*2 + s2b
    #   rows 0..111: k = (s1', s2b', g) -> w_g[c, g] iff s1'==s1, s2b'==s2b
    #   rows 112..127: k = 112 + (c', s2b') -> 1 iff c'==c, s2b''==s2b
    bd = np.zeros((128, 128), np.float32)
    for s1 in range(S):
        for s2b in range(2):
            for g in range(GDIM):
                row = s1 * 14 + s2b * 7 + g
                for c in range(CIN):
                    bd[row, s1 * 16 + c * 2 + s2b] = w_g[c, g]
    for c in range(CIN):
        for s2b in range(2):
            row = 112 + c * 2 + s2b
            for s1 in range(S):
                bd[row, s1 * 16 + c * 2 + s2b] = 1.0

    # per-batch tables, cols (s2a, j) = s2a*128 + j
    # y[b] (j, s2, c) -> (c, s2b, s2a, j)
    y_t = y.transpose(0, 3, 2, 1).reshape(B, CIN, 2, 4, N)      # b,c,s2b,s2a,j
    # ytbl [128, 512]: row (s1, c, s2b), replicated over s1
    ytbl = np.broadcast_to(y_t.reshape(B, 1, CIN, 2, 4 * N),
                           (B, S, CIN, 2, 4 * N)).reshape(B, 128, BW)
    # keytbl [16, 512]: row (c, s2b): w0[c]*y + logmask
    lm_t = logmask.transpose(0, 2, 1).reshape(B, 1, 2, 4, N)    # b,1,s2b,s2a,j
    kt = (w0[None, :, None, None, None] * y_t + lm_t).reshape(B, 16, BW)

    # G^T per block [112, 512]: row (s1, s2b, g), col (s2a, j)
    in_maps = []
    for k in range(NCORES):
        sl = slice(ISHARD * k, ISHARD * (k + 1))
        pg = pairwise_g[:, sl]                      # (B, 16, N, S, S, G)
        arr = pg.reshape(B, ISHARD, N, S, 2, 4, GDIM)
        arr = arr.transpose(0, 1, 3, 4, 6, 5, 2)    # b,i,s1,s2b,g,s2a,j
        arr = arr.reshape(NBLK, KROWS, BW)

        blob = np.empty((128, TOTW), NPBF16)
        blob[:, BD0:BD0 + 128] = bd.astype(NPBF16)
        blob[:, YT0:G0] = ytbl.transpose(1, 0, 2).reshape(128, B * BW)
        gdst = blob[:, G0:].reshape(128, NBLK, BW)
        gdst[:KROWS] = arr.transpose(1, 0, 2)
        gdst[KROWS:] = np.repeat(kt, ISHARD, axis=0).transpose(1, 0, 2)
        in_maps.append({"blob": blob})
    return in_maps


def _host_finish(s_list, coset_functions, mask, w_lin):
    """Decode per-core (128, 128) den/num buffers into the full result."""
    y = np.asarray(coset_functions, dtype=np.float32)
    maskf = np.asarray(mask).astype(np.float32)
    out = np.empty((B, N, S, COUT), np.float32)
    for k in range(NCORES):
        s = s_list[k].astype(np.float32)            # [128, 2*NBLK]
        # rows p = (s1, c, s2b); col 2*blk = den, 2*blk+1 = num
        den = s[:, 0::2].reshape(S, CIN, 2, NBLK).sum(axis=2)   # (s1, c, blk)
        num = s[:, 1::2].reshape(S, CIN, 2, NBLK).sum(axis=2)
        ratio = (num / den).transpose(2, 0, 1)      # (blk, s1, c)
        sl = slice(ISHARD * k, ISHARD * (k + 1))
        y_q = y[:, sl].reshape(NBLK, S, CIN)
        m_q = maskf[:, sl].reshape(NBLK, S)
        res = (y_q + ratio) * m_q[..., None]
        res = res @ w_lin.T
        out[:, sl] = res.reshape(B, ISHARD, S, COUT)
    return out


def kernel(pairwise_g, coset_functions, mask, w_y, b_y, w_g, b_g, w_lin):
    pairwise_g = np.asarray(pairwise_g, dtype=np.float32)
    coset_functions = np.asarray(coset_functions, dtype=np.float32)
    mask = np.asarray(mask)
    w_y = np.asarray(w_y, dtype=np.float32)
    w_g = np.asarray(w_g, dtype=np.float32)
    w_lin = np.asarray(w_lin, dtype=np.float32)

    nc = _get_program()
    in_maps = _host_prep(pairwise_g, coset_functions, mask, w_y, w_g)
    res = run_bass_kernel_spmd(nc, in_maps, core_ids=list(range(NCORES)))
    s_list = [r["out_s"] for r in res.results]
    return _host_finish(s_list, coset_functions, mask, w_lin)
